# revision 8
# baseline (speedup 1.0000x reference)
"""Trainium2 Bass kernel for nn_CMAE_8856222564944 (retrieval_knn).

Computation (reference):
    h = L2-normalize rows of x            [B, N_ITEMS]
    h = tanh(h @ W1 + b1)                 [B, 600]
    h = tanh(h @ W2 + b2)                 [B, 200]
    h = tanh(h @ W3 + b3)                 [B, 600]
    dist = |h|^2 - 2 h @ E^T + |E|^2      [B, N_ITEMS]

Distribution (8 cores, tensor-parallel over the item dim):
    - x^T, W1, E^T are sharded over items (rows of W1/x^T, cols of E^T).
    - Each core computes a partial u^T = W1_sh^T x_sh^T; one AllReduce of
      the small [600, B] hidden; the W2/W3 layers are replicated.
    - Each core computes its column shard of dist and the host concatenates.

Everything ships as bf16 (fp32 PSUM accumulation); the epilogue
(|h|^2, |E|^2 terms) is folded into the dist GEMM via two extra
contraction rows.  Host precomputes: row norms of x (x is pre-normalized
in fp32 on host), -2*E^T, |E|^2, and pads items 50000 -> 50176 = 8*6272.
"""

import sys

if "/opt/trn_rl_repo" not in sys.path:
    sys.path.insert(0, "/opt/trn_rl_repo")

import numpy as np
import ml_dtypes

import concourse.bass as bass
import concourse.mybir as mybir
import concourse.tile as tile
from concourse import bacc

BF16 = ml_dtypes.bfloat16
P = 128

# Full-size problem config
N_CORES = 8
B = 1024
H1 = 600
H2 = 200
N_ITEMS = 50000
ITEMS_PAD = 50176          # 8 * 6272, 6272 = 49 * 128
SH = ITEMS_PAD // N_CORES  # per-core item shard


def _chunks(total, size):
    """[(start, length), ...] covering [0, total) in `size` steps."""
    return [(s, min(size, total - s)) for s in range(0, total, size)]


def build_program(b=B, h1=H1, h2=H2, sh=SH, n_cores=N_CORES):
    """Build the per-core SPMD Bass program (same graph on every core)."""
    dt = mybir.dt
    fp32 = dt.float32
    bf16 = dt.bfloat16

    assert sh % P == 0
    k1 = sh // P                      # item K-tiles for phase 1
    mch = _chunks(h1, P)              # H1 row subtiles: [(0,128)..(512,88)]
    m2ch = _chunks(h2, P)             # H2 row subtiles: [(0,128),(128,72)]
    bch = _chunks(b, 512)             # B column halves (psum free dim)
    nch = _chunks(sh, 448)            # dist output column tiles
    kd = h1 + 2                       # dist contraction rows (h, h_sq, 1)
    # dist K-subtiles: reuse the H1 subtiles; last one grows by 2 rows
    kdch = list(mch[:-1]) + [(mch[-1][0], mch[-1][1] + 2)]
    assert kdch[-1][1] <= P

    nc = bacc.Bacc(
        "TRN2",
        target_bir_lowering=False,
        debug=False,
        enable_asserts=False,
        num_devices=n_cores,
    )

    xT = nc.dram_tensor("xT", [sh, b], bf16, kind="ExternalInput")
    W1d = nc.dram_tensor("W1s", [sh, h1], bf16, kind="ExternalInput")
    W2d = nc.dram_tensor("W2s", [h1, h2], bf16, kind="ExternalInput")
    W3d = nc.dram_tensor("W3s", [h2, h1], bf16, kind="ExternalInput")
    b1d = nc.dram_tensor("b1", [h1], fp32, kind="ExternalInput")
    b2d = nc.dram_tensor("b2", [h2], fp32, kind="ExternalInput")
    b3d = nc.dram_tensor("b3", [h1], fp32, kind="ExternalInput")
    eTd = nc.dram_tensor("eT", [kd, sh], bf16, kind="ExternalInput")
    outd = nc.dram_tensor("dist", [b, sh], fp32, kind="ExternalOutput")

    Tanh = mybir.ActivationFunctionType.Tanh
    Square = mybir.ActivationFunctionType.Square
    rg = [list(range(n_cores))]

    with tile.TileContext(nc) as tc:
        with (
            tc.tile_pool(name="persist", bufs=1) as persist,
            tc.tile_pool(name="dram", bufs=1, space="DRAM") as dram,
            tc.tile_pool(name="xs", bufs=4) as xs_pool,
            tc.tile_pool(name="psum", bufs=1, space="PSUM") as psum_pool,
            tc.tile_pool(name="outs", bufs=4) as out_pool,
        ):
            # ---- persistent SBUF tensors -------------------------------
            W1_sb = persist.tile([P, k1, h1], bf16, name="W1_sb")
            e_sb = persist.tile([P, len(kdch), sh], bf16, name="e_sb")
            W2_sb = persist.tile([P, len(mch), h2], bf16, name="W2_sb")
            W3_sb = persist.tile([P, len(m2ch), h1], bf16, name="W3_sb")
            b1_sb = persist.tile([P, len(mch), 1], fp32, name="b1_sb")
            b2_sb = persist.tile([P, len(m2ch), 1], fp32, name="b2_sb")
            b3_sb = persist.tile([P, len(mch), 1], fp32, name="b3_sb")
            ones_sb = persist.tile([P, len(mch), 1], bf16, name="ones_sb")
            up_sb = persist.tile([P, len(mch), b], fp32, name="up_sb")
            h1_sb = persist.tile([P, len(mch), b], bf16, name="h1_sb")
            h2_sb = persist.tile([P, len(m2ch), b], bf16, name="h2_sb")
            hhat_sb = persist.tile([P, len(kdch), b], bf16, name="hhat_sb")
            hq_sb = persist.tile([1, b], bf16, name="hq_sb")
            one_row_sb = persist.tile([1, b], bf16, name="one_row_sb")

            # small-constant loads
            for ki, (m0, ml) in enumerate(mch):
                nc.sync.dma_start(
                    b1_sb[:ml, ki, :], b1d[m0 : m0 + ml].rearrange("(p o) -> p o", o=1)
                )
                nc.sync.dma_start(
                    b3_sb[:ml, ki, :], b3d[m0 : m0 + ml].rearrange("(p o) -> p o", o=1)
                )
                nc.sync.dma_start(W2_sb[:ml, ki, :], W2d[m0 : m0 + ml, :])
            for ki, (m0, ml) in enumerate(m2ch):
                nc.sync.dma_start(
                    b2_sb[:ml, ki, :], b2d[m0 : m0 + ml].rearrange("(p o) -> p o", o=1)
                )
                nc.sync.dma_start(W3_sb[:ml, ki, :], W3d[m0 : m0 + ml, :])
            nc.vector.memset(ones_sb[:], 1.0)
            nc.vector.memset(one_row_sb[:], 1.0)

            # ---- phase 1: partial u^T = W1_sh^T @ x_sh^T ----------------
            u_bounce = []
            u_red = []
            for hi, (c0, cl) in enumerate(bch):
                u_bounce.append(
                    dram.tile([h1, cl], fp32, name=f"u_bounce{hi}")
                )
                u_red.append(
                    dram.tile(
                        [h1, cl],
                        fp32,
                        addr_space="Shared" if n_cores > 4 else "Local",
                        name=f"u_red{hi}",
                    )
                )

            for hi, (c0, cl) in enumerate(bch):
                psums = [
                    psum_pool.tile([P, 512], fp32, name=f"p1_{hi}_{mi}", tag=f"pbank{mi}")
                    for mi in range(len(mch))
                ]
                for k in range(k1):
                    if hi == 0:
                        nc.sync.dma_start(
                            W1_sb[:, k, :], W1d[k * P : (k + 1) * P, :]
                        )
                    xt = xs_pool.tile([P, cl], bf16, name=f"xt_{hi}_{k}", tag="xt")
                    nc.sync.dma_start(xt[:], xT[k * P : (k + 1) * P, c0 : c0 + cl])
                    for mi, (m0, ml) in enumerate(mch):
                        nc.tensor.matmul(
                            psums[mi][:ml, :cl],
                            W1_sb[:, k, m0 : m0 + ml],
                            xt[:],
                            start=(k == 0),
                            stop=(k == k1 - 1),
                        )
                for mi, (m0, ml) in enumerate(mch):
                    nc.scalar.copy(up_sb[:ml, mi, c0 : c0 + cl], psums[mi][:ml, :cl])
                    nc.sync.dma_start(
                        u_bounce[hi][m0 : m0 + ml, :], up_sb[:ml, mi, c0 : c0 + cl]
                    )
                nc.gpsimd.collective_compute(
                    "AllReduce",
                    mybir.AluOpType.add,
                    replica_groups=rg,
                    ins=[u_bounce[hi].opt()],
                    outs=[u_red[hi].opt()],
                )

            # item_emb^T shard (+aug rows) — emitted after phase 1 so its
            # DMAs don't delay the first matmuls; still lands well before
            # dist needs it.
            for ki, (r0, rl) in enumerate(kdch):
                nc.sync.dma_start(e_sb[:rl, ki, :], eTd[r0 : r0 + rl, :])

            # ---- per-B-half tail: tanh -> W2 -> W3 -> h_sq -> dist ------
            # Half 0's work (incl. its dist quarter) overlaps half 1's
            # AllReduce, keeping the PE warm and hiding collective latency.
            last_k = len(kdch) - 1
            hrow = mch[-1][1]          # h_sq partition within last subtile
            n_m = b // P
            group_sz = 4
            ngroups = [nch[i : i + group_sz] for i in range(0, len(nch), group_sz)]

            for hi, (c0, cl) in enumerate(bch):
                # h1 = tanh(u + b1), cast bf16
                for mi, (m0, ml) in enumerate(mch):
                    nc.sync.dma_start(
                        up_sb[:ml, mi, c0 : c0 + cl], u_red[hi][m0 : m0 + ml, :]
                    )
                    nc.scalar.activation(
                        h1_sb[:ml, mi, c0 : c0 + cl],
                        up_sb[:ml, mi, c0 : c0 + cl],
                        Tanh,
                        bias=b1_sb[:ml, mi, 0:1],
                    )
                # phase 2 (uses pbank5/6 — free while phase-1 h1 accumulates)
                for mi, (m0, ml) in enumerate(m2ch):
                    ps = psum_pool.tile([P, 512], fp32, name=f"p2_{hi}_{mi}", tag=f"pbank{5 + mi}")
                    for k, (r0, rl) in enumerate(mch):
                        nc.tensor.matmul(
                            ps[:ml, :cl],
                            W2_sb[:rl, k, m0 : m0 + ml],
                            h1_sb[:rl, k, c0 : c0 + cl],
                            start=(k == 0),
                            stop=(k == len(mch) - 1),
                        )
                    nc.scalar.activation(
                        h2_sb[:ml, mi, c0 : c0 + cl],
                        ps[:ml, :cl],
                        Tanh,
                        bias=b2_sb[:ml, mi, 0:1],
                    )
                # phase 3 (alternates pbank5/6)
                for mi, (m0, ml) in enumerate(mch):
                    ps = psum_pool.tile([P, 512], fp32, name=f"p3_{hi}_{mi}", tag=f"pbank{5 + mi % 2}")
                    for k, (r0, rl) in enumerate(m2ch):
                        nc.tensor.matmul(
                            ps[:ml, :cl],
                            W3_sb[:rl, k, m0 : m0 + ml],
                            h2_sb[:rl, k, c0 : c0 + cl],
                            start=(k == 0),
                            stop=(k == len(m2ch) - 1),
                        )
                    nc.scalar.activation(
                        hhat_sb[:ml, mi, c0 : c0 + cl],
                        ps[:ml, :cl],
                        Tanh,
                        bias=b3_sb[:ml, mi, 0:1],
                    )
                # h_sq = sum over partitions of hhat^2 (squares into h1_sb,
                # which is dead for these columns after phase 2)
                for ki, (m0, ml) in enumerate(mch):
                    nc.scalar.activation(
                        h1_sb[:ml, ki, c0 : c0 + cl],
                        hhat_sb[:ml, ki, c0 : c0 + cl],
                        Square,
                    )
                psq = psum_pool.tile([1, 512], fp32, name=f"pq_{hi}", tag="pbank7")
                for k, (m0, ml) in enumerate(mch):
                    nc.tensor.matmul(
                        psq[:1, :cl],
                        ones_sb[:ml, k, 0:1],
                        h1_sb[:ml, k, c0 : c0 + cl],
                        start=(k == 0),
                        stop=(k == len(mch) - 1),
                    )
                nc.scalar.copy(hq_sb[0:1, c0 : c0 + cl], psq[:1, :cl])
                # aug rows (partitions 88/89 need DMA, not compute engines)
                nc.sync.dma_start(
                    hhat_sb[hrow : hrow + 1, last_k, c0 : c0 + cl],
                    hq_sb[0:1, c0 : c0 + cl],
                )
                nc.sync.dma_start(
                    hhat_sb[hrow + 1 : hrow + 2, last_k, c0 : c0 + cl],
                    one_row_sb[0:1, c0 : c0 + cl],
                )

                # dist rows for this half's B columns
                for mi in range(c0 // P, (c0 + cl) // P):
                    for gi, grp in enumerate(ngroups):
                        pss = [
                            psum_pool.tile(
                                [P, 512], fp32, name=f"p4_{mi}_{gi}_{j}",
                                tag=f"pbank{(gi % 2) * 4 + j}",
                            )
                            for j in range(len(grp))
                        ]
                        for k, (r0, rl) in enumerate(kdch):
                            for j, (n0, nl) in enumerate(grp):
                                nc.tensor.matmul(
                                    pss[j][:P, :nl],
                                    hhat_sb[:rl, k, mi * P : (mi + 1) * P],
                                    e_sb[:rl, k, n0 : n0 + nl],
                                    start=(k == 0),
                                    stop=(k == len(kdch) - 1),
                                )
                        for j, (n0, nl) in enumerate(grp):
                            ot = out_pool.tile([P, 448], fp32, name=f"ot_{mi}_{gi}_{j}", tag="ot")
                            if j % 2 == 0:
                                nc.scalar.copy(ot[:, :nl], pss[j][:P, :nl])
                            else:
                                nc.vector.tensor_copy(ot[:, :nl], pss[j][:P, :nl])
                            nc.sync.dma_start(
                                outd[mi * P : (mi + 1) * P, n0 : n0 + nl], ot[:, :nl]
                            )

    nc.compile()
    return nc


# ---------------------------------------------------------------------------
# Host side
# ---------------------------------------------------------------------------

def prep_inputs(x, W1, b1, W2, b2, W3, b3, item_emb, n_cores=N_CORES,
                items_pad=ITEMS_PAD):
    """Normalize/cast/transpose/pad/shard the full inputs -> per-core in_maps."""
    n_items = x.shape[1]
    b = x.shape[0]
    h1 = W1.shape[1]
    sh = items_pad // n_cores

    x = np.asarray(x, np.float32)
    norm = np.sqrt((x * x).sum(axis=1, keepdims=True))
    xn = x / np.maximum(norm, 1e-12)

    xT = np.zeros((items_pad, b), dtype=BF16)
    xT[:n_items] = xn.T.astype(BF16)
    W1p = np.zeros((items_pad, h1), dtype=BF16)
    W1p[:n_items] = np.asarray(W1, np.float32).astype(BF16)
    eT = np.zeros((h1 + 2, items_pad), dtype=BF16)
    E = np.asarray(item_emb, np.float32)
    eT[:h1, :n_items] = (-2.0 * E).T.astype(BF16)
    eT[h1, :] = np.ones((items_pad,), dtype=BF16)
    eT[h1 + 1, :n_items] = (E * E).sum(axis=1).astype(BF16)

    common = {
        "W2s": np.ascontiguousarray(np.asarray(W2, np.float32).astype(BF16)),
        "W3s": np.ascontiguousarray(np.asarray(W3, np.float32).astype(BF16)),
        "b1": np.asarray(b1, np.float32),
        "b2": np.asarray(b2, np.float32),
        "b3": np.asarray(b3, np.float32),
    }
    in_maps = []
    for c in range(n_cores):
        in_maps.append(
            dict(
                common,
                xT=np.ascontiguousarray(xT[c * sh : (c + 1) * sh]),
                W1s=np.ascontiguousarray(W1p[c * sh : (c + 1) * sh]),
                eT=np.ascontiguousarray(eT[:, c * sh : (c + 1) * sh]),
            )
        )
    return in_maps


_NC_CACHE = {}


def get_nc():
    if "nc" not in _NC_CACHE:
        _NC_CACHE["nc"] = build_program()
    return _NC_CACHE["nc"]


def kernel(x, W1, b1, W2, b2, W3, b3, item_emb, **run_kwargs):
    from concourse.bass_utils import run_bass_kernel_spmd

    n_items = x.shape[1]
    in_maps = prep_inputs(x, W1, b1, W2, b2, W3, b3, item_emb)
    nc = get_nc()
    res = run_bass_kernel_spmd(nc, in_maps, core_ids=list(range(N_CORES)), **run_kwargs)
    dist = np.concatenate(
        [res.results[c]["dist"] for c in range(N_CORES)], axis=1
    )[:, :n_items]
    if run_kwargs:
        kernel.last_results = res
    return np.ascontiguousarray(dist.astype(np.float32))


# revision 10
# speedup vs baseline: 1.0130x; 1.0130x over previous
"""Trainium2 Bass kernel for nn_CMAE_8856222564944 (retrieval_knn).

Computation (reference):
    h = L2-normalize rows of x            [B, N_ITEMS]
    h = tanh(h @ W1 + b1)                 [B, 600]
    h = tanh(h @ W2 + b2)                 [B, 200]
    h = tanh(h @ W3 + b3)                 [B, 600]
    dist = |h|^2 - 2 h @ E^T + |E|^2      [B, N_ITEMS]

Distribution (8 cores, tensor-parallel over the item dim):
    - x^T, W1, E^T are sharded over items (rows of W1/x^T, cols of E^T).
    - Each core computes a partial u^T = W1_sh^T x_sh^T; one AllReduce of
      the small [600, B] hidden; the W2/W3 layers are replicated.
    - Each core computes its column shard of dist and the host concatenates.

Everything ships as bf16 (fp32 PSUM accumulation); the epilogue
(|h|^2, |E|^2 terms) is folded into the dist GEMM via two extra
contraction rows.  Host precomputes: row norms of x (x is pre-normalized
in fp32 on host), -2*E^T, |E|^2, and pads items 50000 -> 50176 = 8*6272.
"""

import sys

if "/opt/trn_rl_repo" not in sys.path:
    sys.path.insert(0, "/opt/trn_rl_repo")

import numpy as np
import ml_dtypes

import concourse.bass as bass
import concourse.mybir as mybir
import concourse.tile as tile
from concourse import bacc

BF16 = ml_dtypes.bfloat16
P = 128

# Full-size problem config
N_CORES = 8
B = 1024
H1 = 600
H2 = 200
N_ITEMS = 50000
ITEMS_PAD = 50176          # 8 * 6272, 6272 = 49 * 128
SH = ITEMS_PAD // N_CORES  # per-core item shard


def _chunks(total, size):
    """[(start, length), ...] covering [0, total) in `size` steps."""
    return [(s, min(size, total - s)) for s in range(0, total, size)]


def build_program(b=B, h1=H1, h2=H2, sh=SH, n_cores=N_CORES):
    """Build the per-core SPMD Bass program (same graph on every core)."""
    dt = mybir.dt
    fp32 = dt.float32
    bf16 = dt.bfloat16

    assert sh % P == 0
    k1 = sh // P                      # item K-tiles for phase 1
    mch = _chunks(h1, P)              # H1 row subtiles: [(0,128)..(512,88)]
    m2ch = _chunks(h2, P)             # H2 row subtiles: [(0,128),(128,72)]
    bch = _chunks(b, 512)             # B column halves (psum free dim)
    nch = _chunks(sh, 448)            # dist output column tiles
    kd = h1 + 2                       # dist contraction rows (h, h_sq, 1)
    # dist K-subtiles: reuse the H1 subtiles; last one grows by 2 rows
    kdch = list(mch[:-1]) + [(mch[-1][0], mch[-1][1] + 2)]
    assert kdch[-1][1] <= P

    nc = bacc.Bacc(
        "TRN2",
        target_bir_lowering=False,
        debug=False,
        enable_asserts=False,
        num_devices=n_cores,
    )

    xT = nc.dram_tensor("xT", [sh, b], bf16, kind="ExternalInput")
    W1d = nc.dram_tensor("W1s", [sh, h1], bf16, kind="ExternalInput")
    W2d = nc.dram_tensor("W2s", [h1, h2], bf16, kind="ExternalInput")
    W3d = nc.dram_tensor("W3s", [h2, h1], bf16, kind="ExternalInput")
    b1d = nc.dram_tensor("b1", [h1], fp32, kind="ExternalInput")
    b2d = nc.dram_tensor("b2", [h2], fp32, kind="ExternalInput")
    b3d = nc.dram_tensor("b3", [h1], fp32, kind="ExternalInput")
    eTd = nc.dram_tensor("eT", [kd, sh], bf16, kind="ExternalInput")
    outd = nc.dram_tensor("dist", [b, sh], fp32, kind="ExternalOutput")

    Tanh = mybir.ActivationFunctionType.Tanh
    Square = mybir.ActivationFunctionType.Square
    rg = [list(range(n_cores))]

    with tile.TileContext(nc) as tc:
        with (
            tc.tile_pool(name="persist", bufs=1) as persist,
            tc.tile_pool(name="dram", bufs=1, space="DRAM") as dram,
            tc.tile_pool(name="xs", bufs=6) as xs_pool,
            tc.tile_pool(name="psum", bufs=1, space="PSUM") as psum_pool,
            tc.tile_pool(name="outs", bufs=4) as out_pool,
        ):
            # ---- persistent SBUF tensors -------------------------------
            W1_sb = persist.tile([P, k1, h1], bf16, name="W1_sb")
            e_sb = persist.tile([P, len(kdch), sh], bf16, name="e_sb")
            W2_sb = persist.tile([P, len(mch), h2], bf16, name="W2_sb")
            W3_sb = persist.tile([P, len(m2ch), h1], bf16, name="W3_sb")
            b1_sb = persist.tile([P, len(mch), 1], fp32, name="b1_sb")
            b2_sb = persist.tile([P, len(m2ch), 1], fp32, name="b2_sb")
            b3_sb = persist.tile([P, len(mch), 1], fp32, name="b3_sb")
            ones_sb = persist.tile([P, len(mch), 1], bf16, name="ones_sb")
            up_sb = persist.tile([P, len(mch), b], fp32, name="up_sb")
            h1_sb = persist.tile([P, len(mch), b], bf16, name="h1_sb")
            h2_sb = persist.tile([P, len(m2ch), b], bf16, name="h2_sb")
            hhat_sb = persist.tile([P, len(kdch), b], bf16, name="hhat_sb")
            hq_sb = persist.tile([1, b], bf16, name="hq_sb")
            one_row_sb = persist.tile([1, b], bf16, name="one_row_sb")

            nc.vector.memset(ones_sb[:], 1.0)
            nc.vector.memset(one_row_sb[:], 1.0)

            # ---- phase 1: partial u^T = W1_sh^T @ x_sh^T ----------------
            u_bounce = []
            u_red = []
            for hi, (c0, cl) in enumerate(bch):
                u_bounce.append(
                    dram.tile([h1, cl], fp32, name=f"u_bounce{hi}")
                )
                u_red.append(
                    dram.tile(
                        [h1, cl],
                        fp32,
                        addr_space="Shared" if n_cores > 4 else "Local",
                        name=f"u_red{hi}",
                    )
                )

            for hi, (c0, cl) in enumerate(bch):
                psums = [
                    psum_pool.tile([P, 512], fp32, name=f"p1_{hi}_{mi}", tag=f"pbank{mi}")
                    for mi in range(len(mch))
                ]
                for k in range(k1):
                    if hi == 0:
                        nc.scalar.dma_start(
                            W1_sb[:, k, :], W1d[k * P : (k + 1) * P, :]
                        )
                    xt = xs_pool.tile([P, cl], bf16, name=f"xt_{hi}_{k}", tag="xt")
                    nc.sync.dma_start(xt[:], xT[k * P : (k + 1) * P, c0 : c0 + cl])
                    for mi, (m0, ml) in enumerate(mch):
                        nc.tensor.matmul(
                            psums[mi][:ml, :cl],
                            W1_sb[:, k, m0 : m0 + ml],
                            xt[:],
                            start=(k == 0),
                            stop=(k == k1 - 1),
                        )
                for mi, (m0, ml) in enumerate(mch):
                    nc.scalar.copy(up_sb[:ml, mi, c0 : c0 + cl], psums[mi][:ml, :cl])
                    nc.sync.dma_start(
                        u_bounce[hi][m0 : m0 + ml, :], up_sb[:ml, mi, c0 : c0 + cl]
                    )
                nc.gpsimd.collective_compute(
                    "AllReduce",
                    mybir.AluOpType.add,
                    replica_groups=rg,
                    ins=[u_bounce[hi].opt()],
                    outs=[u_red[hi].opt()],
                )

            # item_emb^T shard (+aug rows) — emitted after phase 1 so its
            # DMAs don't delay the first matmuls; still lands well before
            # dist needs it.
            for ki, (r0, rl) in enumerate(kdch):
                nc.sync.dma_start(e_sb[:rl, ki, :], eTd[r0 : r0 + rl, :])
            # small constants — needed only after the first AllReduce lands
            for ki, (m0, ml) in enumerate(mch):
                nc.gpsimd.dma_start(
                    b1_sb[:ml, ki, :], b1d[m0 : m0 + ml].rearrange("(p o) -> p o", o=1)
                )
                nc.gpsimd.dma_start(
                    b3_sb[:ml, ki, :], b3d[m0 : m0 + ml].rearrange("(p o) -> p o", o=1)
                )
                nc.gpsimd.dma_start(W2_sb[:ml, ki, :], W2d[m0 : m0 + ml, :])
            for ki, (m0, ml) in enumerate(m2ch):
                nc.gpsimd.dma_start(
                    b2_sb[:ml, ki, :], b2d[m0 : m0 + ml].rearrange("(p o) -> p o", o=1)
                )
                nc.gpsimd.dma_start(W3_sb[:ml, ki, :], W3d[m0 : m0 + ml, :])

            # ---- per-B-half tail: tanh -> W2 -> W3 -> h_sq -> dist ------
            # Half 0's work (incl. its dist quarter) overlaps half 1's
            # AllReduce, keeping the PE warm and hiding collective latency.
            last_k = len(kdch) - 1
            hrow = mch[-1][1]          # h_sq partition within last subtile
            n_m = b // P
            group_sz = 3
            ngroups = [nch[i : i + group_sz] for i in range(0, len(nch), group_sz)]

            for hi, (c0, cl) in enumerate(bch):
                # h1 = tanh(u + b1), cast bf16
                for mi, (m0, ml) in enumerate(mch):
                    nc.sync.dma_start(
                        up_sb[:ml, mi, c0 : c0 + cl], u_red[hi][m0 : m0 + ml, :]
                    )
                    nc.scalar.activation(
                        h1_sb[:ml, mi, c0 : c0 + cl],
                        up_sb[:ml, mi, c0 : c0 + cl],
                        Tanh,
                        bias=b1_sb[:ml, mi, 0:1],
                    )
                # phase 2 (uses pbank5/6 — free while phase-1 h1 accumulates)
                for mi, (m0, ml) in enumerate(m2ch):
                    ps = psum_pool.tile([P, 512], fp32, name=f"p2_{hi}_{mi}", tag=f"pbank{6 + mi}")
                    for k, (r0, rl) in enumerate(mch):
                        nc.tensor.matmul(
                            ps[:ml, :cl],
                            W2_sb[:rl, k, m0 : m0 + ml],
                            h1_sb[:rl, k, c0 : c0 + cl],
                            start=(k == 0),
                            stop=(k == len(mch) - 1),
                        )
                    nc.scalar.activation(
                        h2_sb[:ml, mi, c0 : c0 + cl],
                        ps[:ml, :cl],
                        Tanh,
                        bias=b2_sb[:ml, mi, 0:1],
                    )
                # phase 3 (alternates pbank5/6)
                for mi, (m0, ml) in enumerate(mch):
                    ps = psum_pool.tile([P, 512], fp32, name=f"p3_{hi}_{mi}", tag=f"pbank{6 + mi % 2}")
                    for k, (r0, rl) in enumerate(m2ch):
                        nc.tensor.matmul(
                            ps[:ml, :cl],
                            W3_sb[:rl, k, m0 : m0 + ml],
                            h2_sb[:rl, k, c0 : c0 + cl],
                            start=(k == 0),
                            stop=(k == len(m2ch) - 1),
                        )
                    nc.scalar.activation(
                        hhat_sb[:ml, mi, c0 : c0 + cl],
                        ps[:ml, :cl],
                        Tanh,
                        bias=b3_sb[:ml, mi, 0:1],
                    )
                # h_sq = sum over partitions of hhat^2 (squares into h1_sb,
                # which is dead for these columns after phase 2)
                for ki, (m0, ml) in enumerate(mch):
                    nc.scalar.activation(
                        h1_sb[:ml, ki, c0 : c0 + cl],
                        hhat_sb[:ml, ki, c0 : c0 + cl],
                        Square,
                    )
                psq = psum_pool.tile([1, 512], fp32, name=f"pq_{hi}", tag="pbank6")
                for k, (m0, ml) in enumerate(mch):
                    nc.tensor.matmul(
                        psq[:1, :cl],
                        ones_sb[:ml, k, 0:1],
                        h1_sb[:ml, k, c0 : c0 + cl],
                        start=(k == 0),
                        stop=(k == len(mch) - 1),
                    )
                nc.scalar.copy(hq_sb[0:1, c0 : c0 + cl], psq[:1, :cl])
                # aug rows (partitions 88/89 need DMA, not compute engines)
                nc.sync.dma_start(
                    hhat_sb[hrow : hrow + 1, last_k, c0 : c0 + cl],
                    hq_sb[0:1, c0 : c0 + cl],
                )
                nc.sync.dma_start(
                    hhat_sb[hrow + 1 : hrow + 2, last_k, c0 : c0 + cl],
                    one_row_sb[0:1, c0 : c0 + cl],
                )

            # dist emitted after BOTH halves' small phases so half 1's
            # tanh/W2/W3 chain executes under dist(half 0)'s matmuls.
            for hi, (c0, cl) in enumerate(bch):
                for mi in range(c0 // P, (c0 + cl) // P):
                    for gi, grp in enumerate(ngroups):
                        pss = [
                            psum_pool.tile(
                                [P, 512], fp32, name=f"p4_{mi}_{gi}_{j}",
                                tag=f"pbank{(gi % 2) * 3 + j}",
                            )
                            for j in range(len(grp))
                        ]
                        for k, (r0, rl) in enumerate(kdch):
                            for j, (n0, nl) in enumerate(grp):
                                nc.tensor.matmul(
                                    pss[j][:P, :nl],
                                    hhat_sb[:rl, k, mi * P : (mi + 1) * P],
                                    e_sb[:rl, k, n0 : n0 + nl],
                                    start=(k == 0),
                                    stop=(k == len(kdch) - 1),
                                )
                        for j, (n0, nl) in enumerate(grp):
                            ot = out_pool.tile([P, 448], fp32, name=f"ot_{mi}_{gi}_{j}", tag="ot")
                            if j % 2 == 0:
                                nc.scalar.copy(ot[:, :nl], pss[j][:P, :nl])
                            else:
                                nc.vector.tensor_copy(ot[:, :nl], pss[j][:P, :nl])
                            nc.sync.dma_start(
                                outd[mi * P : (mi + 1) * P, n0 : n0 + nl], ot[:, :nl]
                            )

    nc.compile()
    return nc


# ---------------------------------------------------------------------------
# Host side
# ---------------------------------------------------------------------------

def prep_inputs(x, W1, b1, W2, b2, W3, b3, item_emb, n_cores=N_CORES,
                items_pad=ITEMS_PAD):
    """Normalize/cast/transpose/pad/shard the full inputs -> per-core in_maps."""
    n_items = x.shape[1]
    b = x.shape[0]
    h1 = W1.shape[1]
    sh = items_pad // n_cores

    x = np.asarray(x, np.float32)
    norm = np.sqrt((x * x).sum(axis=1, keepdims=True))
    xn = x / np.maximum(norm, 1e-12)

    xT = np.zeros((items_pad, b), dtype=BF16)
    xT[:n_items] = xn.T.astype(BF16)
    W1p = np.zeros((items_pad, h1), dtype=BF16)
    W1p[:n_items] = np.asarray(W1, np.float32).astype(BF16)
    eT = np.zeros((h1 + 2, items_pad), dtype=BF16)
    E = np.asarray(item_emb, np.float32)
    eT[:h1, :n_items] = (-2.0 * E).T.astype(BF16)
    eT[h1, :] = np.ones((items_pad,), dtype=BF16)
    eT[h1 + 1, :n_items] = (E * E).sum(axis=1).astype(BF16)

    common = {
        "W2s": np.ascontiguousarray(np.asarray(W2, np.float32).astype(BF16)),
        "W3s": np.ascontiguousarray(np.asarray(W3, np.float32).astype(BF16)),
        "b1": np.asarray(b1, np.float32),
        "b2": np.asarray(b2, np.float32),
        "b3": np.asarray(b3, np.float32),
    }
    in_maps = []
    for c in range(n_cores):
        in_maps.append(
            dict(
                common,
                xT=np.ascontiguousarray(xT[c * sh : (c + 1) * sh]),
                W1s=np.ascontiguousarray(W1p[c * sh : (c + 1) * sh]),
                eT=np.ascontiguousarray(eT[:, c * sh : (c + 1) * sh]),
            )
        )
    return in_maps


_NC_CACHE = {}


def get_nc():
    if "nc" not in _NC_CACHE:
        _NC_CACHE["nc"] = build_program()
    return _NC_CACHE["nc"]


def kernel(x, W1, b1, W2, b2, W3, b3, item_emb, **run_kwargs):
    from concourse.bass_utils import run_bass_kernel_spmd

    n_items = x.shape[1]
    in_maps = prep_inputs(x, W1, b1, W2, b2, W3, b3, item_emb)
    nc = get_nc()
    res = run_bass_kernel_spmd(nc, in_maps, core_ids=list(range(N_CORES)), **run_kwargs)
    dist = np.concatenate(
        [res.results[c]["dist"] for c in range(N_CORES)], axis=1
    )[:, :n_items]
    if run_kwargs:
        kernel.last_results = res
    return np.ascontiguousarray(dist.astype(np.float32))
